# revision 1
# baseline (speedup 1.0000x reference)
"""3-layer GAT (PPI-style) forward on 8 Trainium2 NeuronCores.

Strategy (SPMD, one NEFF on 8 cores):
  - Host: add self-loops, degree-balanced node permutation into 8 cores x
    2500 nodes (tiles of 128 dst nodes), edges sorted by dst and padded to a
    uniform chunk count; int16 gather-index arrays precomputed.
  - Per layer: sharded dense phase (PE): [h | lin] = x @ [W | Wl] (bf16),
    es/ed attention dots via DVE; payload row [h0|1|h1|1|...|es|ed] (bf16 +
    f32 tail); AllGather payload across cores; aggregation phase: dma_gather
    payload[src] per edge + 256B ed[dst] gather, attention weights
    w = exp(max(t, 0.2t)) (exact softmax, no max-subtraction needed: |t|<~8),
    per-head one-hot x weight lhsT via one fused tensor_scalar, PE matmul
    accumulates segment sums + denominator (trailing ones column), normalize,
    add skip + bias, ELU -> next layer input (bf16 rows).
"""

import math
import numpy as np

N_CORES = 8
GROUP = 6  # gather chunks per dma_gather group


# --------------------------------------------------------------------------
# host-side prep (pure data layout / graph partitioning, no model math)
# --------------------------------------------------------------------------

def _balance_permutation(dst, n, n_cores, tiles_per_core, rows_last):
    """Greedy balance: nodes -> 128-row dst tiles with ~equal edge counts."""
    import heapq

    deg = np.bincount(dst, minlength=n).astype(np.int64)
    order = np.argsort(-deg, kind="stable")
    n_tiles = n_cores * tiles_per_core
    caps = np.full(n_tiles, 128, np.int64)
    caps[tiles_per_core - 1 :: tiles_per_core] = rows_last
    heap = [(0, int(b)) for b in range(n_tiles)]
    heapq.heapify(heap)
    members = [[] for _ in range(n_tiles)]
    loads = np.zeros(n_tiles, np.int64)
    for node in order:
        while True:
            load, b = heapq.heappop(heap)
            if len(members[b]) < caps[b]:
                break
        members[b].append(node)
        loads[b] += deg[node]
        if len(members[b]) < caps[b]:
            heapq.heappush(heap, (int(loads[b]), b))
    perm_o2n = np.empty(n, np.int64)
    per_core = tiles_per_core * 128 - (128 - rows_last)
    for b in range(n_tiles):
        core, t = divmod(b, tiles_per_core)
        base = core * per_core + t * 128
        ids = np.asarray(members[b], np.int64)
        perm_o2n[ids] = base + np.arange(len(ids))
    return perm_o2n


def _wrap16_rep(a):
    """[L] int -> [128, L/16] int16 (16-wrap, replicated 8x down partitions)."""
    w = a.reshape(-1, 16).T.astype(np.int16)
    return np.ascontiguousarray(np.tile(w, (8, 1)))


def _host_prep(inputs, n_cores=N_CORES):
    import ml_dtypes

    bf16 = ml_dtypes.bfloat16
    x = np.asarray(inputs["x"], np.float32)
    ei = np.asarray(inputs["edge_index"])
    n, f_in = x.shape
    loop = np.arange(n, dtype=ei.dtype)
    src = np.concatenate([ei[0], loop]).astype(np.int64)
    dst = np.concatenate([ei[1], loop]).astype(np.int64)

    per_core = n // n_cores
    tiles_per_core = math.ceil(per_core / 128)
    rows_last = per_core - (tiles_per_core - 1) * 128

    perm = _balance_permutation(dst, n, n_cores, tiles_per_core, rows_last)
    src_n = perm[src]
    dst_n = perm[dst]

    core_of = dst_n // per_core
    # per (core, tile) edge lists
    counts = np.zeros((n_cores, tiles_per_core), np.int64)
    per_ct_src = {}
    per_ct_dst = {}
    per_ct_loc = {}
    for c in range(n_cores):
        sel = core_of == c
        s, d = src_n[sel], dst_n[sel]
        loc = d - c * per_core
        o = np.argsort(loc, kind="stable")
        s, d, loc = s[o], d[o], loc[o]
        tile_of = loc // 128
        for t in range(tiles_per_core):
            m = tile_of == t
            per_ct_src[c, t] = s[m]
            per_ct_dst[c, t] = d[m]
            per_ct_loc[c, t] = loc[m] - t * 128
            counts[c, t] = m.sum()

    nchunk = math.ceil(counts.max() / 128)
    group = min(GROUP, nchunk)
    nchunk = math.ceil(nchunk / group) * group

    cap = nchunk * 128
    src16 = np.zeros((n_cores, tiles_per_core, 128, nchunk * 8), np.int16)
    dst16 = np.zeros((n_cores, tiles_per_core, 128, nchunk * 8), np.int16)
    dstloc = np.full((n_cores, tiles_per_core, 128, nchunk), -1.0, np.float32)
    for c in range(n_cores):
        for t in range(tiles_per_core):
            e = counts[c, t]
            ps = np.zeros(cap, np.int64)
            pd = np.zeros(cap, np.int64)
            pl = np.full(cap, -1.0, np.float32)
            ps[:e] = per_ct_src[c, t]
            pd[:e] = per_ct_dst[c, t]
            pl[:e] = per_ct_loc[c, t]
            src16[c, t] = _wrap16_rep(ps)
            dst16[c, t] = _wrap16_rep(pd)
            dstloc[c, t] = pl.reshape(nchunk, 128).T

    # permuted node features, transposed, padded rows, bf16, per core
    rows_pad = tiles_per_core * 128
    x_perm = np.zeros((n, f_in), np.float32)
    x_perm[perm] = x
    xT = []
    for c in range(n_cores):
        blk = np.zeros((rows_pad, f_in), np.float32)
        blk[:per_core] = x_perm[c * per_core : (c + 1) * per_core]
        xT.append(np.ascontiguousarray(blk.T).astype(bf16))

    g = lambda k: np.asarray(inputs[k], np.float32)
    waug1 = np.concatenate([g("W1"), g("Wl1")], 1).astype(bf16)   # [50, 2048]
    waug2 = np.concatenate([g("W2"), g("Wl2")], 1).astype(bf16)   # [1024, 2048]
    waug3 = np.concatenate([g("W3"), g("Wl3")], 1).astype(bf16)   # [1024, 847]

    rep = lambda v: np.ascontiguousarray(np.broadcast_to(v[None, :], (128, v.shape[0])))
    a_flat = lambda k: rep(g(k).reshape(-1)).astype(np.float32)

    base = dict(
        waug1=waug1, waug2=waug2, waug3=waug3,
        a1s=a_flat("a1s"), a1d=a_flat("a1d"),
        a2s=a_flat("a2s"), a2d=a_flat("a2d"),
        a3s=a_flat("a3s"), a3d=a_flat("a3d"),
        b1=rep(g("b1")), bl1=rep(g("bl1")),
        b2=rep(g("b2")), bl2=rep(g("bl2")),
        b3=rep(g("b3")), bl3=rep(g("bl3")),
    )
    in_maps = []
    for c in range(n_cores):
        m = dict(base)
        m["xT1"] = xT[c]
        m["src16"] = src16[c]
        m["dst16"] = dst16[c]
        m["dstloc"] = dstloc[c]
        in_maps.append(m)

    h1, c1 = np.asarray(inputs["a1s"]).shape
    h3, c3 = np.asarray(inputs["a3s"]).shape
    d1 = h1 * c1
    cfg = dict(
        n=n, f_in=f_in, n_cores=n_cores, per_core=per_core,
        tiles_per_core=tiles_per_core, rows_last=rows_last, rows_pad=rows_pad,
        nchunk=nchunk, group=group,
        h1=h1, c1=c1, d1=d1, h3=h3, c3=c3,
    )
    return in_maps, cfg, perm


# --------------------------------------------------------------------------
# bass program
# --------------------------------------------------------------------------

def _layer_dims(cfg):
    """Static per-layer dims. Payload row (bf16 units):
    [h0 | 1 | h1 | 1 | ... ] (H*(C+1)) then es (H f32), ed (H f32), pad."""
    out = []
    for li in (1, 2, 3):
        if li < 3:
            h, c = cfg["h1"], cfg["c1"]
            din = cfg["f_in"] if li == 1 else cfg["d1"]
            naug = cfg["d1"] * 2
            nlin = cfg["d1"]
        else:
            h, c = cfg["h3"], cfg["c3"]
            din = cfg["d1"]
            naug = cfg["h3"] * cfg["c3"] + cfg["c3"]
            nlin = cfg["c3"]
        st = c + 1
        hst = h * st
        es = math.ceil(hst / 2)             # f32 offset of es
        ed = es + h
        pw_f32 = math.ceil((ed + h) / 64) * 64
        pw = pw_f32 * 2                     # payload bf16 width
        eds = min((ed // 64) * 64, pw_f32 - 64)
        assert ed + h - eds <= 64 and es >= eds or True
        kch = math.ceil(din / 128)
        out.append(dict(li=li, din=din, kch=kch, naug=naug, nlin=nlin,
                        h=h, c=c, st=st, hst=hst, es=es, ed=ed,
                        pw=pw, pw_f32=pw_f32, eds=eds))
    return out


def _build(cfg):
    import concourse.bass as bass
    import concourse.bacc as bacc
    import concourse.mybir as mybir
    import concourse.tile as tile
    from contextlib import ExitStack

    f32 = mybir.dt.float32
    bf = mybir.dt.bfloat16
    i16 = mybir.dt.int16
    i32 = mybir.dt.int32
    u16 = mybir.dt.uint16
    EXP = mybir.ActivationFunctionType.Exp
    ALU = mybir.AluOpType

    n_cores = cfg["n_cores"]
    n = cfg["n"]
    T = cfg["tiles_per_core"]
    rows_last = cfg["rows_last"]
    per_core = cfg["per_core"]
    rows_pad = cfg["rows_pad"]
    NCHUNK = cfg["nchunk"]
    GRP = cfg["group"]
    NG = NCHUNK // GRP
    D1 = cfg["d1"]
    layers = _layer_dims(cfg)
    PWMAX = max(L["pw"] for L in layers)
    HMAX = max(L["h"] for L in layers)

    nc = bacc.Bacc(None, target_bir_lowering=False)

    # ---- parameters -----------------------------------------------------
    xT1 = nc.declare_dram_parameter("xT1", [cfg["f_in"], rows_pad], bf, isOutput=False)
    waug_p = {
        1: nc.declare_dram_parameter("waug1", [cfg["f_in"], layers[0]["naug"]], bf, isOutput=False),
        2: nc.declare_dram_parameter("waug2", [D1, layers[1]["naug"]], bf, isOutput=False),
        3: nc.declare_dram_parameter("waug3", [D1, layers[2]["naug"]], bf, isOutput=False),
    }
    a_p = {}
    for li, L in ((1, layers[0]), (2, layers[1]), (3, layers[2])):
        a_p[li, "s"] = nc.declare_dram_parameter(f"a{li}s", [128, L["h"] * L["c"]], f32, isOutput=False)
        a_p[li, "d"] = nc.declare_dram_parameter(f"a{li}d", [128, L["h"] * L["c"]], f32, isOutput=False)
    b_p = {}
    for li, L in ((1, layers[0]), (2, layers[1]), (3, layers[2])):
        b_p[li, "b"] = nc.declare_dram_parameter(f"b{li}", [128, L["nlin"]], f32, isOutput=False)
        b_p[li, "l"] = nc.declare_dram_parameter(f"bl{li}", [128, L["nlin"]], f32, isOutput=False)
    src16_p = nc.declare_dram_parameter("src16", [T, 128, NCHUNK * 8], i16, isOutput=False)
    dst16_p = nc.declare_dram_parameter("dst16", [T, 128, NCHUNK * 8], i16, isOutput=False)
    dstloc_p = nc.declare_dram_parameter("dstloc", [T, 128, NCHUNK], f32, isOutput=False)
    out_p = nc.declare_dram_parameter("out", [per_core, cfg["c3"]], f32, isOutput=True)

    with tile.TileContext(nc, num_cores=n_cores) as tc, ExitStack() as ctx:
        # ---- dram scratch ----------------------------------------------
        dram = ctx.enter_context(tc.tile_pool(name="dram", bufs=1, space="DRAM"))
        pshard = {L["li"]: dram.tile([per_core, L["pw"]], u16, tag=f"pshard{L['li']}", name=f"pshard{L['li']}")
                  for L in layers}
        pfull = {L["li"]: dram.tile([n, L["pw"]], u16, tag=f"pfull{L['li']}", name=f"pfull{L['li']}",
                                    addr_space="Shared") for L in layers}
        xrows = {li: dram.tile([rows_pad, D1], bf, tag=f"xrows{li}", name=f"xrows{li}") for li in (1, 2)}
        linb = {L["li"]: dram.tile([rows_pad, L["nlin"]], f32, tag=f"lin{L['li']}", name=f"lin{L['li']}")
                for L in layers}

        # ---- pools ------------------------------------------------------
        consts = ctx.enter_context(tc.tile_pool(name="consts", bufs=1))
        waugp = ctx.enter_context(tc.tile_pool(name="waugp", bufs=1))
        xtp = ctx.enter_context(tc.tile_pool(name="xtp", bufs=4))
        ptp = ctx.enter_context(tc.tile_pool(name="ptp", bufs=3))
        ltp = ctx.enter_context(tc.tile_pool(name="ltp", bufs=2))
        tmpp = ctx.enter_context(tc.tile_pool(name="tmpp", bufs=2))
        esp = ctx.enter_context(tc.tile_pool(name="esp", bufs=2))
        idxp = ctx.enter_context(tc.tile_pool(name="idxp", bufs=2))
        edp = ctx.enter_context(tc.tile_pool(name="edp", bufs=3))
        gp = ctx.enter_context(tc.tile_pool(name="gp", bufs=3))
        lgp = ctx.enter_context(tc.tile_pool(name="lgp", bufs=3))
        ohwp = ctx.enter_context(tc.tile_pool(name="ohwp", bufs=6))
        epip = ctx.enter_context(tc.tile_pool(name="epip", bufs=2))
        recp = ctx.enter_context(tc.tile_pool(name="recp", bufs=8))
        abufp = ctx.enter_context(tc.tile_pool(name="abufp", bufs=1))
        bsump = ctx.enter_context(tc.tile_pool(name="bsump", bufs=1))
        psum_d = ctx.enter_context(tc.tile_pool(name="psum_d", bufs=1, space="PSUM"))
        psum_a = ctx.enter_context(tc.tile_pool(name="psum_a", bufs=1, space="PSUM"))

        # ---- constants ---------------------------------------------------
        iota_i = consts.tile([128, 128], i32, tag="iota_i")
        nc.gpsimd.iota(iota_i[:, :], pattern=[[1, 128]], base=0, channel_multiplier=0)
        iota_f = consts.tile([128, 128], f32, tag="iota_f")
        nc.vector.tensor_copy(iota_f[:, :], iota_i[:, :])

        def rows_of(t):
            return 128 if t < T - 1 else rows_last

        # ------------------------------------------------------------------
        for L in layers:
            li, DIN, KCH, NAUG = L["li"], L["din"], L["kch"], L["naug"]
            H, C, ST, HST = L["h"], L["c"], L["st"], L["hst"]
            PW, PWF, ES, ED, EDS = L["pw"], L["pw_f32"], L["es"], L["ed"], L["eds"]
            NLIN = L["nlin"]
            HC = H * C

            # layer constants
            wt = [waugp.tile([128, NAUG], bf, tag=f"waug_kc{k}", name=f"waug_kc{k}") for k in range(KCH)]
            for k in range(KCH):
                kk = min(128, DIN - k * 128)
                nc.sync.dma_start(out=wt[k][:kk, :], in_=waug_p[li][k * 128 : k * 128 + kk, :])
            a_s = abufp.tile([128, HC], f32, tag="a_s")
            a_d = abufp.tile([128, HC], f32, tag="a_d")
            nc.sync.dma_start(out=a_s[:, :], in_=a_p[li, "s"][:, :])
            nc.sync.dma_start(out=a_d[:, :], in_=a_p[li, "d"][:, :])
            bsum = bsump.tile([128, NLIN], f32, tag="bsum")
            btmp = bsump.tile([128, NLIN], f32, tag="btmp")
            nc.sync.dma_start(out=bsum[:, :], in_=b_p[li, "b"][:, :])
            nc.sync.dma_start(out=btmp[:, :], in_=b_p[li, "l"][:, :])
            nc.vector.tensor_tensor(out=bsum[:, :], in0=bsum[:, :], in1=btmp[:, :], op=ALU.add)

            # ---------------- dense phase --------------------------------
            two_pass = NAUG > 1024
            wA = HC if two_pass else NAUG

            def load_lhsT(t, k, kk):
                lhsT = xtp.tile([128, 128], bf, tag="lhsT", name="lhsT")
                if li == 1:
                    nc.sync.dma_start(out=lhsT[:kk, :], in_=xT1[:, t * 128 : (t + 1) * 128])
                else:
                    nc.sync.dma_start(
                        out=lhsT[:, :],
                        in_=xrows[li - 1][t * 128 : (t + 1) * 128, k * 128 : (k + 1) * 128],
                        transpose=True,
                    )
                return lhsT

            for t in range(T):
                pdA = psum_d.tile([128, wA], f32, tag="pd", name="pdA")
                for k in range(KCH):
                    kk = min(128, DIN - k * 128)
                    lhsT = load_lhsT(t, k, kk)
                    for nb in range(math.ceil(wA / 512)):
                        w = min(512, wA - nb * 512)
                        nc.tensor.matmul(
                            pdA[:, nb * 512 : nb * 512 + w],
                            lhsT[:kk, :],
                            wt[k][:kk, nb * 512 : nb * 512 + w],
                            start=(k == 0),
                            stop=(k == KCH - 1),
                        )
                # es/ed
                est = esp.tile([128, H], f32, tag="est")
                edt = esp.tile([128, H], f32, tag="edt")
                for which, avec, dstt in (("s", a_s, est), ("d", a_d, edt)):
                    tmp = tmpp.tile([128, HC], f32, tag="tmp", name="tmp")
                    nc.vector.tensor_tensor(out=tmp[:, :], in0=pdA[:, :HC], in1=avec[:, :], op=ALU.mult)
                    nc.vector.reduce_sum(
                        dstt[:, :], tmp.rearrange("p (h c) -> p h c", h=H),
                        axis=mybir.AxisListType.X,
                    )
                # payload assembly
                pt = ptp.tile([128, PW], u16, tag="pt")
                ptb = pt.bitcast(bf)
                for h in range(H):
                    nc.vector.tensor_copy(ptb[:, h * ST : h * ST + C], pdA[:, h * C : (h + 1) * C])
                    nc.vector.memset(ptb[:, h * ST + C : h * ST + C + 1], 1.0)
                ptf = pt.bitcast(f32)
                nc.vector.tensor_copy(ptf[:, ES : ES + H], est[:, :])
                nc.vector.tensor_copy(ptf[:, ED : ED + H], edt[:, :])
                if 2 * (ED + H) < PW:
                    nc.vector.memset(pt[:, 2 * (ED + H) : PW], 0.0)
                r = rows_of(t)
                nc.sync.dma_start(out=pshard[li][t * 128 : t * 128 + r, :], in_=pt[:r, :])
                # lin + bias staging
                lt = ltp.tile([128, NLIN], f32, tag="lt")
                if two_pass:
                    pdB = psum_d.tile([128, NLIN], f32, tag="pd", name="pdB")
                    for k in range(KCH):
                        kk = min(128, DIN - k * 128)
                        lhsT = load_lhsT(t, k, kk)
                        for nb in range(math.ceil(NLIN / 512)):
                            w = min(512, NLIN - nb * 512)
                            nc.tensor.matmul(
                                pdB[:, nb * 512 : nb * 512 + w],
                                lhsT[:kk, :],
                                wt[k][:kk, HC + nb * 512 : HC + nb * 512 + w],
                                start=(k == 0),
                                stop=(k == KCH - 1),
                            )
                    nc.vector.tensor_tensor(out=lt[:, :], in0=pdB[:, :], in1=bsum[:, :], op=ALU.add)
                else:
                    nc.vector.tensor_tensor(out=lt[:, :], in0=pdA[:, HC : HC + NLIN], in1=bsum[:, :], op=ALU.add)
                nc.sync.dma_start(out=linb[li][t * 128 : t * 128 + r, :], in_=lt[:r, :])

            # ---------------- all-gather ---------------------------------
            nc.gpsimd.collective_compute(
                "AllGather",
                ALU.bypass,
                replica_groups=[list(range(n_cores))],
                ins=[pshard[li].opt()],
                outs=[pfull[li].opt()],
            )

            pfull_f = pfull[li].bitcast(f32)

            # ---------------- aggregation phase --------------------------
            for t in range(T):
                r = rows_of(t)
                s16 = idxp.tile([128, NCHUNK * 8], i16, tag="s16")
                d16 = idxp.tile([128, NCHUNK * 8], i16, tag="d16")
                dloc = idxp.tile([128, NCHUNK], f32, tag="dloc")
                nc.sync.dma_start(out=s16[:, :], in_=src16_p[t])
                nc.sync.dma_start(out=d16[:, :], in_=dst16_p[t])
                nc.sync.dma_start(out=dloc[:, :], in_=dstloc_p[t])

                ps = [psum_a.tile([128, ST], f32, tag=f"ps{h}", name=f"ps{h}") for h in range(H)]
                psl = [(ps[h], 0) for h in range(H)]

                for g in range(NG):
                    G = gp.tile([128, GRP, PW], u16, tag="G")
                    nc.gpsimd.dma_gather(
                        out_ap=G[:, :, :],
                        in_ap=pfull[li][:, :],
                        idxs_ap=s16[:, g * GRP * 8 : (g + 1) * GRP * 8],
                        num_idxs=GRP * 128,
                        num_idxs_reg=GRP * 128,
                        elem_size=PW,
                    )
                    edg = edp.tile([128, GRP, 64], f32, tag="edg")
                    nc.gpsimd.dma_gather(
                        out_ap=edg[:, :, :],
                        in_ap=pfull_f[:, EDS : EDS + 64],
                        idxs_ap=d16[:, g * GRP * 8 : (g + 1) * GRP * 8],
                        num_idxs=GRP * 128,
                        num_idxs_reg=GRP * 128,
                        elem_size=64,
                        elem_step=PWF,
                    )
                    Gf = G.bitcast(f32)
                    Gb = G.bitcast(bf)
                    tl = lgp.tile([128, GRP, H], f32, tag="tl")
                    t2 = lgp.tile([128, GRP, H], f32, tag="t2")
                    wf = lgp.tile([128, GRP, H], f32, tag="wf")
                    we = lgp.tile([128, GRP, H], f32, tag="we")
                    nc.vector.tensor_tensor(
                        out=tl[:, :, :], in0=Gf[:, :, ES : ES + H],
                        in1=edg[:, :, ED - EDS : ED - EDS + H],
                        op=ALU.add,
                    )
                    nc.vector.tensor_scalar(out=t2[:, :, :], in0=tl[:, :, :],
                                            scalar1=0.2, scalar2=None, op0=ALU.mult)
                    nc.vector.tensor_tensor(out=wf[:, :, :], in0=tl[:, :, :], in1=t2[:, :, :], op=ALU.max)
                    nc.scalar.activation(we[:, :, :], wf[:, :, :], EXP)
                    for cch in range(GRP):
                        j = g * GRP + cch
                        for h in range(H):
                            ohw = ohwp.tile([128, 128], bf, tag="ohw")
                            nc.vector.tensor_scalar(
                                out=ohw[:, :], in0=iota_f[:, :],
                                scalar1=dloc[:, j : j + 1],
                                scalar2=we[:, cch, h : h + 1],
                                op0=ALU.is_equal, op1=ALU.mult,
                            )
                            pst, off = psl[h]
                            nc.tensor.matmul(
                                pst[:, off : off + ST],
                                ohw[:, :],
                                Gb[:, cch, h * ST : (h + 1) * ST],
                                start=(j == 0),
                                stop=(j == NCHUNK - 1),
                            )

                # epilogue
                xt = epip.tile([128, HC], f32, tag="xt")
                for h in range(H):
                    pst, off = psl[h]
                    rec = recp.tile([128, 1], f32, tag="rec")
                    nc.vector.reciprocal(rec[:, :], pst[:, off + C : off + C + 1])
                    nc.vector.tensor_scalar(
                        out=xt[:, h * C : (h + 1) * C], in0=pst[:, off : off + C],
                        scalar1=rec[:, 0:1], scalar2=None, op0=ALU.mult,
                    )
                lt2 = ltp.tile([128, NLIN], f32, tag="lt2")
                nc.sync.dma_start(out=lt2[:r, :], in_=linb[li][t * 128 : t * 128 + r, :])
                if li < 3:
                    s = epip.tile([128, HC], f32, tag="s")
                    u = epip.tile([128, HC], f32, tag="u")
                    e = epip.tile([128, HC], f32, tag="e")
                    v = epip.tile([128, HC], f32, tag="v")
                    xo = epip.tile([128, HC], bf, tag="xo")
                    nc.vector.tensor_tensor(out=s[:r, :], in0=xt[:r, :], in1=lt2[:r, :], op=ALU.add)
                    nc.vector.tensor_scalar(out=u[:r, :], in0=s[:r, :], scalar1=0.0, scalar2=None, op0=ALU.min)
                    nc.scalar.activation(e[:r, :], u[:r, :], EXP)
                    nc.vector.tensor_scalar(out=v[:r, :], in0=s[:r, :], scalar1=0.0, scalar2=-1.0,
                                            op0=ALU.max, op1=ALU.add)
                    nc.vector.tensor_tensor(out=xo[:r, :], in0=v[:r, :], in1=e[:r, :], op=ALU.add)
                    nc.sync.dma_start(out=xrows[li][t * 128 : t * 128 + r, :], in_=xo[:r, :])
                else:
                    xt3 = xt.rearrange("p (h c) -> p h c", h=H)
                    m1 = epip.tile([128, 3, C], f32, tag="m1")
                    nc.vector.tensor_tensor(out=m1[:, :, :], in0=xt3[:, 0:3, :], in1=xt3[:, 3:6, :], op=ALU.add)
                    m2 = epip.tile([128, C], f32, tag="m2")
                    nc.vector.tensor_tensor(out=m2[:, :], in0=m1[:, 0, :], in1=m1[:, 1, :], op=ALU.add)
                    m3 = epip.tile([128, C], f32, tag="m3")
                    nc.vector.tensor_tensor(out=m3[:, :], in0=m2[:, :], in1=m1[:, 2, :], op=ALU.add)
                    ot = epip.tile([128, C], f32, tag="ot")
                    nc.vector.tensor_scalar(out=ot[:r, :], in0=m3[:r, :], scalar1=1.0 / H,
                                            scalar2=None, op0=ALU.mult)
                    nc.vector.tensor_tensor(out=ot[:r, :], in0=ot[:r, :], in1=lt2[:r, :], op=ALU.add)
                    nc.sync.dma_start(out=out_p[t * 128 : t * 128 + r, :], in_=ot[:r, :])

    nc.finalize()
    return nc


# --------------------------------------------------------------------------
# runner
# --------------------------------------------------------------------------

def _run(inputs, sim=False, trace=False, n_cores=N_CORES, tmpdir=None):
    in_maps, cfg, perm = _host_prep(inputs, n_cores)
    nc = _build(cfg)
    if sim:
        import concourse.bass_interp as bass_interp

        msim = bass_interp.MultiCoreSim(nc, n_cores)
        for c in range(n_cores):
            for k, v in in_maps[c].items():
                msim.cores[c].tensor(k)[:] = v
        msim.simulate(check_with_hw=True)
        outs = [np.array(msim.cores[c].mem_tensor("out")) for c in range(n_cores)]
        exec_ns = None
    else:
        from concourse.bass_utils import run_bass_kernel_spmd

        res = run_bass_kernel_spmd(
            nc, in_maps, list(range(n_cores)), trace=trace, tmpdir=tmpdir
        )
        outs = [res.results[c]["out"] for c in range(n_cores)]
        exec_ns = res.exec_time_ns
    out_new = np.concatenate(outs, 0)
    out = np.empty_like(out_new)
    out[...] = out_new[perm]
    return out.astype(np.float32), exec_ns


def kernel(**inputs) -> np.ndarray:
    out, _ = _run(inputs)
    return out



# revision 14
# speedup vs baseline: 1.6044x; 1.6044x over previous
"""3-layer GAT (PPI-style) forward on 8 Trainium2 NeuronCores.

Strategy (SPMD, one NEFF on 8 cores):
  - Host: add self-loops, degree-balanced node permutation into 8 cores x
    20 tiles of 128 dst nodes (rows_pad=2560/core), edges sorted by dst and
    padded to a uniform chunk count; int16 gather-index arrays and STATIC
    per-chunk one-hot (edge -> dst slot) matrices precomputed on host.
  - Attention dots es/ed become extra matmul columns (host precomputes
    ws = W_h @ a_s_h per head), so dense phase emits [h | es | ed | lin]
    in one pass.
  - Payload row = [h (bf16) | es (f32)]; ed is NOT allgathered: it goes to a
    compact local DRAM table gathered by dst (256B rows).
  - AllGather of the payload is chunked (4 x 5 tiles) and overlapped with
    the producing compute; pfull uses a chunk-major layout so each chunked
    collective writes contiguous rows (host remaps gather indices).
  - Aggregation per 128-edge chunk: gather payload rows by src, w =
    exp(leakyrelu(es+ed)) via ACT engine, ONE DVE op scales the payload by
    w (broadcast AP), then two 512-col matmuls accumulate the weighted
    segment sum using the STATIC one-hot as lhsT, plus one tiny matmul for
    the softmax denominator.
  - Layer fusion: epilogue of layer i's aggregation (normalize + skip +
    ELU) immediately feeds layer i+1's dense matmuls for the same tile via
    PE-transposes (no DRAM round trip, no transpose-DMAs).
"""

import math
import numpy as np

N_CORES = 8
GROUP = 6   # gather chunks per dma_gather group
AG_TILES = 5  # tiles per chunked AllGather (20 tiles -> 4 chunks)
AG_LAG = 3    # emit AG trigger this many tiles after its last producer
CHUNKED_AG = True  # 4 overlapped AllGathers (non-Shared out) vs 1 Shared AG


# --------------------------------------------------------------------------
# host-side prep (data layout / graph partitioning / weight packing)
# --------------------------------------------------------------------------

def _balance_permutation(dst, n, n_cores, tiles_per_core, rows_last):
    """Greedy balance: nodes -> 128-row dst tiles with ~equal edge counts."""
    import heapq

    deg = np.bincount(dst, minlength=n).astype(np.int64)
    order = np.argsort(-deg, kind="stable")
    n_tiles = n_cores * tiles_per_core
    caps = np.full(n_tiles, 128, np.int64)
    caps[tiles_per_core - 1 :: tiles_per_core] = rows_last
    heap = [(0, int(b)) for b in range(n_tiles)]
    heapq.heapify(heap)
    members = [[] for _ in range(n_tiles)]
    loads = np.zeros(n_tiles, np.int64)
    for node in order:
        while True:
            load, b = heapq.heappop(heap)
            if len(members[b]) < caps[b]:
                break
        members[b].append(node)
        loads[b] += deg[node]
        if len(members[b]) < caps[b]:
            heapq.heappush(heap, (int(loads[b]), b))
    perm_o2n = np.empty(n, np.int64)
    per_core = tiles_per_core * 128 - (128 - rows_last)
    for b in range(n_tiles):
        core, t = divmod(b, tiles_per_core)
        base = core * per_core + t * 128
        ids = np.asarray(members[b], np.int64)
        perm_o2n[ids] = base + np.arange(len(ids))
    return perm_o2n


def _wrap16_rep(a):
    """[L] int -> [128, L/16] int16 (16-wrap, replicated 8x down partitions)."""
    w = a.reshape(-1, 16).T.astype(np.int16)
    return np.ascontiguousarray(np.tile(w, (8, 1)))


def _host_prep(inputs, n_cores=N_CORES):
    import ml_dtypes

    bf16 = ml_dtypes.bfloat16
    x = np.asarray(inputs["x"], np.float32)
    ei = np.asarray(inputs["edge_index"])
    n, f_in = x.shape
    loop = np.arange(n, dtype=ei.dtype)
    src = np.concatenate([ei[0], loop]).astype(np.int64)
    dst = np.concatenate([ei[1], loop]).astype(np.int64)

    per_core = n // n_cores
    T = math.ceil(per_core / 128)
    rows_last = per_core - (T - 1) * 128
    rows_pad = T * 128

    perm = _balance_permutation(dst, n, n_cores, T, rows_last)
    src_n = perm[src]
    dst_n = perm[dst]

    # chunk-major pfull flat index: node new-id -> AG-chunk-major row
    n_ag = math.ceil(T / AG_TILES)
    ag_rows = AG_TILES * 128  # rows per core per AG chunk

    def flat_of(x_new):
        c = x_new // per_core
        w = x_new % per_core
        t = w // 128
        r = w % 128
        if not CHUNKED_AG:
            return c * rows_pad + t * 128 + r
        k = t // AG_TILES
        return ((k * n_cores + c) * ag_rows + (t - k * AG_TILES) * 128 + r)

    src_flat = flat_of(src_n)

    core_of = dst_n // per_core
    counts = np.zeros((n_cores, T), np.int64)
    per_ct_srcf = {}
    per_ct_dloc = {}   # local row id within core (edtab row), per edge
    per_ct_slot = {}   # dst slot within tile [0,128)
    for c in range(n_cores):
        sel = core_of == c
        sf, d = src_flat[sel], dst_n[sel]
        loc = d - c * per_core
        o = np.argsort(loc, kind="stable")
        sf, loc = sf[o], loc[o]
        tile_of = loc // 128
        for t in range(T):
            m = tile_of == t
            per_ct_srcf[c, t] = sf[m]
            per_ct_dloc[c, t] = loc[m]
            per_ct_slot[c, t] = loc[m] - t * 128
            counts[c, t] = m.sum()

    nchunk = math.ceil(counts.max() / 128)
    group = min(GROUP, nchunk)
    nchunk = math.ceil(nchunk / group) * group
    cap = nchunk * 128

    src16 = np.zeros((n_cores, T, 128, nchunk * 8), np.int16)
    dst16 = np.zeros((n_cores, T, 128, nchunk * 8), np.int16)
    onehot = np.zeros((n_cores, T, 128, nchunk * 128), bf16)
    for c in range(n_cores):
        for t in range(T):
            e = counts[c, t]
            ps = np.zeros(cap, np.int64)
            pd = np.zeros(cap, np.int64)
            sl = np.full(cap, -1, np.int64)
            ps[:e] = per_ct_srcf[c, t]
            pd[:e] = per_ct_dloc[c, t]
            sl[:e] = per_ct_slot[c, t]
            src16[c, t] = _wrap16_rep(ps)
            dst16[c, t] = _wrap16_rep(pd)
            # onehot[p, j*128 + d] = 1 iff edge (j*128+p) has dst slot d
            oh = np.zeros((128, nchunk, 128), np.float32)
            sl2 = sl.reshape(nchunk, 128)  # [j, p]
            jj, pp = np.nonzero(sl2 >= 0)
            oh[pp, jj, sl2[jj, pp]] = 1.0
            onehot[c, t] = oh.reshape(128, nchunk * 128).astype(bf16)

    # permuted node features, transposed, padded rows, bf16, per core
    x_perm = np.zeros((n, f_in), np.float32)
    x_perm[perm] = x
    xT = []
    for c in range(n_cores):
        blk = np.zeros((rows_pad, f_in), np.float32)
        blk[:per_core] = x_perm[c * per_core : (c + 1) * per_core]
        xT.append(np.ascontiguousarray(blk.T).astype(bf16))

    g = lambda k: np.asarray(inputs[k], np.float32)

    # augmented weights: [W | ws | wd | Wl] with ws_h = W_h @ a_s_h etc.
    def aug(W, a_s, a_d, Wl):
        H, C = a_s.shape
        Wh = W.reshape(W.shape[0], H, C)
        ws = np.einsum("dhc,hc->dh", Wh, a_s)
        wd = np.einsum("dhc,hc->dh", Wh, a_d)
        return np.concatenate([W, ws, wd, Wl], 1).astype(bf16)

    waug1 = aug(g("W1"), g("a1s"), g("a1d"), g("Wl1"))   # [50, 2056]
    waug2 = aug(g("W2"), g("a2s"), g("a2d"), g("Wl2"))   # [1024, 2056]
    waug3 = aug(g("W3"), g("a3s"), g("a3d"), g("Wl3"))   # [1024, 859]

    rep = lambda v: np.ascontiguousarray(np.broadcast_to(v[None, :], (128, v.shape[0]))).astype(np.float32)
    base = dict(
        waug1=waug1, waug2=waug2, waug3=waug3,
        bsum1=rep(g("b1") + g("bl1")),
        bsum2=rep(g("b2") + g("bl2")),
        bsum3=rep(g("b3") + g("bl3")),
    )
    in_maps = []
    for c in range(n_cores):
        m = dict(base)
        m["xT1"] = xT[c]
        m["src16"] = src16[c]
        m["dst16"] = dst16[c]
        m["onehot"] = onehot[c]
        in_maps.append(m)

    h1, c1 = np.asarray(inputs["a1s"]).shape
    h3, c3 = np.asarray(inputs["a3s"]).shape
    cfg = dict(
        n=n, f_in=f_in, n_cores=n_cores, per_core=per_core,
        tiles_per_core=T, rows_last=rows_last, rows_pad=rows_pad,
        nchunk=nchunk, group=group, n_ag=n_ag, ag_rows=ag_rows,
        h1=h1, c1=c1, d1=h1 * c1, h3=h3, c3=c3,
    )
    return in_maps, cfg, perm


# --------------------------------------------------------------------------
# bass program
# --------------------------------------------------------------------------

def _layer_dims(cfg):
    """Static per-layer dims.
    waug cols: [h (H*C) | es (H) | ed (H) | lin (NLIN)]
    payload row (u16 units): [h bf16 (HC) | es f32 (2H u16)] padded to PW."""
    out = []
    for li in (1, 2, 3):
        if li < 3:
            h, c = cfg["h1"], cfg["c1"]
            din = cfg["f_in"] if li == 1 else cfg["d1"]
            nlin = cfg["d1"]
        else:
            h, c = cfg["h3"], cfg["c3"]
            din = cfg["d1"]
            nlin = cfg["c3"]
        hc = h * c
        naug = hc + 2 * h + nlin
        pw = math.ceil((hc + 2 * h) / 128) * 128       # u16; 2304B / 1536B rows
        esf = hc // 2                                   # f32 offset of es in payload
        kch = math.ceil(din / 128)
        out.append(dict(li=li, din=din, kch=kch, naug=naug, nlin=nlin,
                        h=h, c=c, hc=hc, pw=pw, esf=esf))
    return out


def _build(cfg):
    import concourse.bass as bass
    import concourse.bacc as bacc
    import concourse.mybir as mybir
    import concourse.tile as tile
    from contextlib import ExitStack

    f32 = mybir.dt.float32
    bf = mybir.dt.bfloat16
    i16 = mybir.dt.int16
    i32 = mybir.dt.int32
    u16 = mybir.dt.uint16
    EXP = mybir.ActivationFunctionType.Exp
    ALU = mybir.AluOpType

    n_cores = cfg["n_cores"]
    T = cfg["tiles_per_core"]
    per_core = cfg["per_core"]
    rows_pad = cfg["rows_pad"]
    NCHUNK = cfg["nchunk"]
    GRP = cfg["group"]
    NG = NCHUNK // GRP
    NAG = cfg["n_ag"]
    AGR = cfg["ag_rows"]
    D1 = cfg["d1"]
    layers = _layer_dims(cfg)
    PWMAX = max(L["pw"] for L in layers)

    nc = bacc.Bacc(None, target_bir_lowering=False)

    # ---- parameters -----------------------------------------------------
    xT1 = nc.declare_dram_parameter("xT1", [cfg["f_in"], rows_pad], bf, isOutput=False)
    waug_p = {L["li"]: nc.declare_dram_parameter(f"waug{L['li']}", [L["din"], L["naug"]], bf, isOutput=False)
              for L in layers}
    bsum_p = {L["li"]: nc.declare_dram_parameter(f"bsum{L['li']}", [128, L["nlin"]], f32, isOutput=False)
              for L in layers}
    src16_p = nc.declare_dram_parameter("src16", [T, 128, NCHUNK * 8], i16, isOutput=False)
    dst16_p = nc.declare_dram_parameter("dst16", [T, 128, NCHUNK * 8], i16, isOutput=False)
    onehot_p = nc.declare_dram_parameter("onehot", [T, 128, NCHUNK * 128], bf, isOutput=False)
    out_p = nc.declare_dram_parameter("out", [per_core, cfg["c3"]], f32, isOutput=True)

    with tile.TileContext(nc, num_cores=n_cores) as tc, ExitStack() as ctx:
        # ---- dram scratch ----------------------------------------------
        dram = ctx.enter_context(tc.tile_pool(name="dram", bufs=1, space="DRAM"))
        pshard = {L["li"]: dram.tile([rows_pad, L["pw"]], u16, tag=f"pshard{L['li']}", name=f"pshard{L['li']}")
                  for L in layers}
        pfull = {L["li"]: dram.tile([n_cores * rows_pad, L["pw"]], u16, tag=f"pfull{L['li']}",
                                    name=f"pfull{L['li']}",
                                    **({} if CHUNKED_AG else dict(addr_space="Shared")))
                 for L in layers}
        linb = {L["li"]: dram.tile([rows_pad, L["nlin"]], f32, tag=f"lin{L['li']}", name=f"lin{L['li']}")
                for L in layers}
        edtab = {L["li"]: dram.tile([rows_pad, 64], f32, tag=f"edtab{L['li']}", name=f"edtab{L['li']}")
                 for L in layers}

        # ---- sbuf pools -------------------------------------------------
        consts = ctx.enter_context(tc.tile_pool(name="consts", bufs=1))
        waugp = ctx.enter_context(tc.tile_pool(name="waugp", bufs=1))
        bsump = ctx.enter_context(tc.tile_pool(name="bsump", bufs=1))
        idxp = ctx.enter_context(tc.tile_pool(name="idxp", bufs=2))
        ohp = ctx.enter_context(tc.tile_pool(name="ohp", bufs=3))
        gp12 = ctx.enter_context(tc.tile_pool(name="gp12", bufs=2))
        gp3 = ctx.enter_context(tc.tile_pool(name="gp3", bufs=2))
        edp = ctx.enter_context(tc.tile_pool(name="edp", bufs=3))
        wep = ctx.enter_context(tc.tile_pool(name="wep", bufs=3))
        rhsp = ctx.enter_context(tc.tile_pool(name="rhsp", bufs=3))
        xtp = ctx.enter_context(tc.tile_pool(name="xtp", bufs=2))
        ptp = ctx.enter_context(tc.tile_pool(name="ptp", bufs=2))
        ltp = ctx.enter_context(tc.tile_pool(name="ltp", bufs=2))
        epip = ctx.enter_context(tc.tile_pool(name="epip", bufs=2))
        recp = ctx.enter_context(tc.tile_pool(name="recp", bufs=2))
        # psum: pfeat 2 banks + psmall 2x1 + pdF 2 + pdL 2 = 8 banks
        pfeatp = ctx.enter_context(tc.tile_pool(name="pfeatp", bufs=1, space="PSUM"))
        psmallp = ctx.enter_context(tc.tile_pool(name="psmallp", bufs=2, space="PSUM"))
        pdFp = ctx.enter_context(tc.tile_pool(name="pdFp", bufs=1, space="PSUM"))
        pdLp = ctx.enter_context(tc.tile_pool(name="pdLp", bufs=1, space="PSUM"))

        # ---- constants --------------------------------------------------
        iota_c = consts.tile([128, 128], i32, tag="iota_c")
        nc.gpsimd.iota(iota_c[:, :], pattern=[[1, 128]], base=0, channel_multiplier=0)
        iota_p = consts.tile([128, 1], i32, tag="iota_p")
        nc.gpsimd.iota(iota_p[:, :], pattern=[[1, 1]], base=0, channel_multiplier=1)
        iota_cf = consts.tile([128, 128], f32, tag="iota_cf")
        nc.vector.tensor_copy(iota_cf[:, :], iota_c[:, :])
        iota_pf = consts.tile([128, 1], f32, tag="iota_pf")
        nc.vector.tensor_copy(iota_pf[:, :], iota_p[:, :])
        ident = consts.tile([128, 128], bf, tag="ident")
        nc.vector.tensor_scalar(out=ident[:, :], in0=iota_cf[:, :],
                                scalar1=iota_pf[:, 0:1], scalar2=None, op0=ALU.is_equal)

        # ---- layer constants (all preloaded; ~51KB/partition) -----------
        wt = {}
        for L in layers:
            li, KCH, DIN, NAUG = L["li"], L["kch"], L["din"], L["naug"]
            wt[li] = [waugp.tile([128, NAUG], bf, tag=f"waug{li}_k{k}", name=f"waug{li}_k{k}")
                      for k in range(KCH)]
            for k in range(KCH):
                kk = min(128, DIN - k * 128)
                nc.sync.dma_start(out=wt[li][k][:kk, :], in_=waug_p[li][k * 128 : k * 128 + kk, :])
        bsum = {}
        for L in layers:
            li = L["li"]
            bsum[li] = bsump.tile([128, L["nlin"]], f32, tag=f"bsum{li}", name=f"bsum{li}")
            nc.sync.dma_start(out=bsum[li][:, :], in_=bsum_p[li][:, :])

        # -----------------------------------------------------------------
        def emit_dense(L, t, lhsT_of):
            """Dense phase for tile t of layer L: psum [h|esd|lin] via matmuls,
            then payload/edtab/linb stores. lhsT_of(k, kk) -> [kk,128] bf AP."""
            li, KCH, DIN = L["li"], L["kch"], L["din"]
            H, HC, NLIN, PW = L["h"], L["hc"], L["nlin"], L["pw"]
            pdF = pdFp.tile([128, 1024], f32, tag="pdF", name="pdF")
            pdE = psmallp.tile([128, 128], f32, tag="ps", name="pdE")
            pdL = pdLp.tile([128, 1024], f32, tag="pdL", name="pdL")
            for k in range(KCH):
                kk = min(128, DIN - k * 128)
                lhsT = lhsT_of(k, kk)
                st, sp = (k == 0), (k == KCH - 1)
                for nb in range(math.ceil(HC / 512)):
                    w = min(512, HC - nb * 512)
                    nc.tensor.matmul(pdF[:, nb * 512 : nb * 512 + w], lhsT,
                                     wt[li][k][:kk, nb * 512 : nb * 512 + w], start=st, stop=sp)
                nc.tensor.matmul(pdE[:, : 2 * H], lhsT,
                                 wt[li][k][:kk, HC : HC + 2 * H], start=st, stop=sp)
                for nb in range(math.ceil(NLIN / 512)):
                    w = min(512, NLIN - nb * 512)
                    nc.tensor.matmul(pdL[:, nb * 512 : nb * 512 + w], lhsT,
                                     wt[li][k][:kk, HC + 2 * H + nb * 512 : HC + 2 * H + nb * 512 + w],
                                     start=st, stop=sp)
            # payload: [h bf16 | es f32 | pad]
            pt = ptp.tile([128, PWMAX], u16, tag="pt")
            ptb = pt.bitcast(bf)
            nc.vector.tensor_copy(ptb[:, :HC], pdF[:, :HC])
            ptf = pt.bitcast(f32)
            nc.vector.tensor_copy(ptf[:, L["esf"] : L["esf"] + H], pdE[:, :H])
            if HC + 2 * H < PW:
                nc.vector.memset(pt[:, HC + 2 * H : PW], 0.0)
            nc.sync.dma_start(out=pshard[li][t * 128 : (t + 1) * 128, :], in_=pt[:, :PW])
            # ed -> local table (row = local node id; full 256B rows initialized)
            edc = recp.tile([128, 64], f32, tag="edc")
            nc.vector.memset(edc[:, H:], 0.0)
            nc.vector.tensor_copy(edc[:, :H], pdE[:, H : 2 * H])
            nc.sync.dma_start(out=edtab[li][t * 128 : (t + 1) * 128, :], in_=edc[:, :])
            # lin + bias
            lt = ltp.tile([128, 1024], f32, tag="lt")
            nc.vector.tensor_tensor(out=lt[:, :NLIN], in0=pdL[:, :NLIN], in1=bsum[li][:, :], op=ALU.add)
            nc.sync.dma_start(out=linb[li][t * 128 : (t + 1) * 128, :], in_=lt[:, :NLIN])

        def emit_ag(L, k):
            """Chunked AllGather k of layer L's payload shard."""
            li = L["li"]
            if not CHUNKED_AG:
                if k == NAG - 1:  # single full AllGather once all tiles stored
                    nc.gpsimd.collective_compute(
                        "AllGather", ALU.bypass,
                        replica_groups=[list(range(n_cores))],
                        ins=[pshard[li][:, :]],
                        outs=[pfull[li][:, :]],
                    )
                return
            a, b = k * AGR, (k + 1) * AGR
            nc.gpsimd.collective_compute(
                "AllGather", ALU.bypass,
                replica_groups=[list(range(n_cores))],
                ins=[pshard[li][a:b, :]],
                outs=[pfull[li][k * n_cores * AGR : (k + 1) * n_cores * AGR, :]],
            )

        # ---------------- phase 1: dense layer 1 -------------------------
        L1 = layers[0]
        f_in = cfg["f_in"]
        for t in range(T):
            x1t = xtp.tile([128, 128], bf, tag="xT_k0", name="x1t")
            nc.sync.dma_start(out=x1t[:f_in, :], in_=xT1[:, t * 128 : (t + 1) * 128])
            emit_dense(L1, t, lambda k, kk: x1t[:f_in, :])
            if (t + 1) % AG_TILES == 0:
                emit_ag(L1, t // AG_TILES)

        # ---------------- phases 2..4: agg(li) [+ dense(li+1)] -----------
        for L in layers:
            li = L["li"]
            H, C, HC, PW, ESF, NLIN = L["h"], L["c"], L["hc"], L["pw"], L["esf"], L["nlin"]
            Lnext = layers[li] if li < 3 else None
            gpool = gp12 if li < 3 else gp3

            for t in range(T):
                # deferred AG trigger for the next layer's payload
                if Lnext is not None and t >= AG_TILES - 1 + AG_LAG and (t - AG_LAG + 1) % AG_TILES == 0:
                    emit_ag(Lnext, (t - AG_LAG + 1) // AG_TILES - 1)

                s16 = idxp.tile([128, NCHUNK * 8], i16, tag="s16")
                d16 = idxp.tile([128, NCHUNK * 8], i16, tag="d16")
                nc.sync.dma_start(out=s16[:, :], in_=src16_p[t])
                nc.sync.dma_start(out=d16[:, :], in_=dst16_p[t])
                lt2 = ltp.tile([128, 1024], f32, tag="lt2")
                nc.sync.dma_start(out=lt2[:, :NLIN], in_=linb[li][t * 128 : (t + 1) * 128, :])

                pfeat = pfeatp.tile([128, 1024], f32, tag="pfeat", name="pfeat")
                pden = psmallp.tile([128, 128], f32, tag="ps", name="pden")

                for g in range(NG):
                    G = gpool.tile([128, GRP, PW], u16, tag=f"G{li if li == 3 else 12}")
                    nc.gpsimd.dma_gather(
                        out_ap=G[:, :, :], in_ap=pfull[li][:, :],
                        idxs_ap=s16[:, g * GRP * 8 : (g + 1) * GRP * 8],
                        num_idxs=GRP * 128, num_idxs_reg=GRP * 128, elem_size=PW,
                    )
                    edg = edp.tile([128, GRP, 64], f32, tag="edg")
                    nc.gpsimd.dma_gather(
                        out_ap=edg[:, :, :], in_ap=edtab[li][:, :],
                        idxs_ap=d16[:, g * GRP * 8 : (g + 1) * GRP * 8],
                        num_idxs=GRP * 128, num_idxs_reg=GRP * 128, elem_size=64,
                    )
                    oh = ohp.tile([128, GRP, 128], bf, tag="oh")
                    nc.sync.dma_start(
                        out=oh[:, :, :],
                        in_=onehot_p[t][:, g * GRP * 128 : (g + 1) * GRP * 128]
                        .rearrange("p (g c) -> p g c", g=GRP),
                    )
                    Gf = G.bitcast(f32)
                    Gb = G.bitcast(bf)
                    tl = wep.tile([128, GRP, H], f32, tag="tl")
                    wf = wep.tile([128, GRP, H], f32, tag="wf")
                    web = wep.tile([128, GRP, H], bf, tag="web")
                    nc.vector.tensor_tensor(out=tl[:, :, :], in0=Gf[:, :, ESF : ESF + H],
                                            in1=edg[:, :, :H], op=ALU.add)
                    # w = exp(max(t, 0.2t)) == exp(leakyrelu(t)); |t| <~ 8
                    nc.vector.tensor_scalar(out=wf[:, :, :], in0=tl[:, :, :],
                                            scalar1=0.2, scalar2=None, op0=ALU.mult)
                    nc.vector.tensor_tensor(out=wf[:, :, :], in0=tl[:, :, :], in1=wf[:, :, :], op=ALU.max)
                    nc.scalar.activation(web[:, :, :], wf[:, :, :], EXP)
                    for cch in range(GRP):
                        j = g * GRP + cch
                        rq = rhsp.tile([128, 1024], bf, tag="rq")
                        nc.vector.tensor_tensor(
                            out=rq[:, :HC].rearrange("p (h c) -> p h c", h=H),
                            in0=Gb[:, cch, :HC].rearrange("p (h c) -> p h c", h=H),
                            in1=web[:, cch, :, None].broadcast_to((128, H, C)),
                            op=ALU.mult,
                        )
                        st, sp = (j == 0), (j == NCHUNK - 1)
                        lhsT = oh[:, cch, :]
                        for nb in range(math.ceil(HC / 512)):
                            w = min(512, HC - nb * 512)
                            nc.tensor.matmul(pfeat[:, nb * 512 : nb * 512 + w], lhsT,
                                             rq[:, nb * 512 : nb * 512 + w], start=st, stop=sp)
                        nc.tensor.matmul(pden[:, :H], lhsT, web[:, cch, :], start=st, stop=sp)

                # ---- epilogue ----
                rec = recp.tile([128, 16], f32, tag="rec")
                nc.vector.tensor_scalar(out=rec[:, 8 : 8 + H], in0=pden[:, :H],
                                        scalar1=1e-16, scalar2=None, op0=ALU.max)
                nc.vector.reciprocal(rec[:, :H], rec[:, 8 : 8 + H])
                if li == 3:
                    nc.vector.tensor_scalar(out=rec[:, :H], in0=rec[:, :H],
                                            scalar1=1.0 / H, scalar2=None, op0=ALU.mult)
                xt = epip.tile([128, 1024], f32, tag="xt")
                nc.vector.tensor_tensor(
                    out=xt[:, :HC].rearrange("p (h c) -> p h c", h=H),
                    in0=pfeat[:, :HC].rearrange("p (h c) -> p h c", h=H),
                    in1=rec[:, :H, None].broadcast_to((128, H, C)),
                    op=ALU.mult,
                )
                r = 128 if t < T - 1 else cfg["rows_last"]
                if li < 3:
                    s = epip.tile([128, 1024], f32, tag="s")
                    u = epip.tile([128, 1024], f32, tag="u")
                    v = epip.tile([128, 1024], f32, tag="v")
                    e = epip.tile([128, 1024], f32, tag="e")
                    xo = epip.tile([128, 1024], bf, tag="xo")
                    nc.vector.tensor_tensor(out=s[:, :], in0=xt[:, :], in1=lt2[:, :], op=ALU.add)
                    nc.vector.tensor_scalar(out=u[:, :], in0=s[:, :], scalar1=0.0, scalar2=None, op0=ALU.min)
                    nc.scalar.activation(e[:, :], u[:, :], EXP)
                    nc.vector.tensor_scalar(out=v[:, :], in0=s[:, :], scalar1=0.0, scalar2=-1.0,
                                            op0=ALU.max, op1=ALU.add)
                    nc.vector.tensor_tensor(out=xo[:, :], in0=v[:, :], in1=e[:, :], op=ALU.add)
                    # ---- fused dense of next layer: PE-transpose lhsT ----
                    xTt = [xtp.tile([128, 128], bf, tag=f"xT_k{k}", name=f"xT_k{k}") for k in range(8)]
                    for k in range(8):
                        tp = psmallp.tile([128, 128], f32, tag="ps", name=f"tp{k}")
                        tpb = tp.bitcast(bf)
                        nc.tensor.transpose(tpb[:, :128], xo[:, k * 128 : (k + 1) * 128], ident[:, :])
                        nc.vector.tensor_copy(xTt[k][:, :], tpb[:, :128])
                    emit_dense(Lnext, t, lambda k, kk: xTt[k][:, :])
                else:
                    x3 = xt[:, :HC].rearrange("p (h c) -> p h c", h=H)
                    m1 = epip.tile([128, 3 * 128], f32, tag="m1")
                    m1v = m1[:, : 3 * C].rearrange("p (h c) -> p h c", h=3)
                    nc.vector.tensor_tensor(out=m1v, in0=x3[:, 0:3, :], in1=x3[:, 3:6, :], op=ALU.add)
                    ot = epip.tile([128, 128], f32, tag="ot")
                    nc.vector.tensor_tensor(out=ot[:, :C], in0=m1[:, :C], in1=m1[:, C : 2 * C], op=ALU.add)
                    nc.vector.tensor_tensor(out=ot[:, :C], in0=ot[:, :C], in1=m1[:, 2 * C : 3 * C], op=ALU.add)
                    nc.vector.tensor_tensor(out=ot[:, :C], in0=ot[:, :C], in1=lt2[:, :C], op=ALU.add)
                    nc.sync.dma_start(out=out_p[t * 128 : t * 128 + r, :], in_=ot[:r, :C])

            # tail AG chunks for the next layer not yet emitted
            if Lnext is not None:
                for k in range((T - AG_LAG) // AG_TILES, NAG):
                    emit_ag(Lnext, k)

    nc.finalize()
    return nc


# --------------------------------------------------------------------------
# runner
# --------------------------------------------------------------------------

def _run(inputs, sim=False, trace=False, n_cores=N_CORES, tmpdir=None):
    in_maps, cfg, perm = _host_prep(inputs, n_cores)
    nc = _build(cfg)
    if sim:
        import concourse.bass_interp as bass_interp

        msim = bass_interp.MultiCoreSim(nc, n_cores)
        for c in range(n_cores):
            for k, v in in_maps[c].items():
                msim.cores[c].tensor(k)[:] = v
        msim.simulate()
        outs = [np.array(msim.cores[c].mem_tensor("out")) for c in range(n_cores)]
        exec_ns = None
    else:
        from concourse.bass_utils import run_bass_kernel_spmd

        res = run_bass_kernel_spmd(
            nc, in_maps, list(range(n_cores)), trace=trace, tmpdir=tmpdir
        )
        outs = [res.results[c]["out"] for c in range(n_cores)]
        exec_ns = res.exec_time_ns
    out_new = np.concatenate(outs, 0)
    out = np.empty_like(out_new)
    out[...] = out_new[perm]
    return out.astype(np.float32), exec_ns


def kernel(**inputs) -> np.ndarray:
    out, _ = _run(inputs)
    return out


# revision 31
# speedup vs baseline: 1.7010x; 1.0602x over previous
"""3-layer GAT (PPI-style) forward on 8 Trainium2 NeuronCores.

Strategy (SPMD, one NEFF on 8 cores):
  - Host: add self-loops, degree-balanced node permutation into 8 cores x
    20 tiles of 128 dst nodes (rows_pad=2560/core), edges sorted by dst and
    padded to a uniform chunk count; int16 gather-index arrays and STATIC
    per-chunk one-hot (edge -> dst slot) matrices precomputed on host.
  - Attention dots es/ed become extra matmul columns (host precomputes
    ws = W_h @ a_s_h per head), so dense phase emits [h | es | ed | lin]
    in one pass.
  - Payload row = [h (bf16) | es (f32)]; ed is NOT allgathered: it goes to a
    compact local DRAM table gathered by dst (256B rows).
  - AllGather of the payload is chunked (4 x 5 tiles) and overlapped with
    the producing compute; pfull uses a chunk-major layout so each chunked
    collective writes contiguous rows (host remaps gather indices).
  - Aggregation per 128-edge chunk: gather payload rows by src, w =
    exp(leakyrelu(es+ed)) via ACT engine, ONE DVE op scales the payload by
    w (broadcast AP), then two 512-col matmuls accumulate the weighted
    segment sum using the STATIC one-hot as lhsT, plus one tiny matmul for
    the softmax denominator.
  - Layer fusion: epilogue of layer i's aggregation (normalize + skip +
    ELU) immediately feeds layer i+1's dense matmuls for the same tile via
    PE-transposes (no DRAM round trip, no transpose-DMAs).
"""

import math
import numpy as np

N_CORES = 8
GROUP = 6   # gather chunks per dma_gather group
AG_TILES = 5  # tiles per chunked AllGather (20 tiles -> 4 chunks)
AG_LAG = 3    # emit AG trigger this many tiles after its last producer
CHUNKED_AG = True  # 4 overlapped AllGathers (non-Shared out) vs 1 Shared AG


# --------------------------------------------------------------------------
# host-side prep (data layout / graph partitioning / weight packing)
# --------------------------------------------------------------------------

def _balance_permutation(dst, n, n_cores, tiles_per_core, rows_last):
    """Greedy balance: nodes -> 128-row dst tiles with ~equal edge counts."""
    import heapq

    deg = np.bincount(dst, minlength=n).astype(np.int64)
    order = np.argsort(-deg, kind="stable")
    n_tiles = n_cores * tiles_per_core
    caps = np.full(n_tiles, 128, np.int64)
    caps[tiles_per_core - 1 :: tiles_per_core] = rows_last
    heap = [(0, int(b)) for b in range(n_tiles)]
    heapq.heapify(heap)
    members = [[] for _ in range(n_tiles)]
    loads = np.zeros(n_tiles, np.int64)
    for node in order:
        while True:
            load, b = heapq.heappop(heap)
            if len(members[b]) < caps[b]:
                break
        members[b].append(node)
        loads[b] += deg[node]
        if len(members[b]) < caps[b]:
            heapq.heappush(heap, (int(loads[b]), b))
    perm_o2n = np.empty(n, np.int64)
    per_core = tiles_per_core * 128 - (128 - rows_last)
    for b in range(n_tiles):
        core, t = divmod(b, tiles_per_core)
        base = core * per_core + t * 128
        ids = np.asarray(members[b], np.int64)
        perm_o2n[ids] = base + np.arange(len(ids))
    return perm_o2n


def _wrap16_rep(a):
    """[L] int -> [128, L/16] int16 (16-wrap, replicated 8x down partitions)."""
    w = a.reshape(-1, 16).T.astype(np.int16)
    return np.ascontiguousarray(np.tile(w, (8, 1)))


def _host_prep(inputs, n_cores=N_CORES):
    import ml_dtypes

    bf16 = ml_dtypes.bfloat16
    x = np.asarray(inputs["x"], np.float32)
    ei = np.asarray(inputs["edge_index"])
    n, f_in = x.shape
    loop = np.arange(n, dtype=ei.dtype)
    src = np.concatenate([ei[0], loop]).astype(np.int64)
    dst = np.concatenate([ei[1], loop]).astype(np.int64)

    per_core = n // n_cores
    T = math.ceil(per_core / 128)
    rows_last = per_core - (T - 1) * 128
    rows_pad = T * 128

    perm = _balance_permutation(dst, n, n_cores, T, rows_last)
    src_n = perm[src]
    dst_n = perm[dst]

    # chunk-major pfull flat index: node new-id -> AG-chunk-major row
    n_ag = math.ceil(T / AG_TILES)
    ag_rows = AG_TILES * 128  # rows per core per AG chunk

    def flat_of(x_new):
        c = x_new // per_core
        w = x_new % per_core
        t = w // 128
        r = w % 128
        if not CHUNKED_AG:
            return c * rows_pad + t * 128 + r
        k = t // AG_TILES
        return ((k * n_cores + c) * ag_rows + (t - k * AG_TILES) * 128 + r)

    src_flat = flat_of(src_n)

    core_of = dst_n // per_core
    counts = np.zeros((n_cores, T), np.int64)
    per_ct_srcf = {}
    per_ct_dloc = {}   # local row id within core (edtab row), per edge
    per_ct_slot = {}   # dst slot within tile [0,128)
    for c in range(n_cores):
        sel = core_of == c
        sf, d = src_flat[sel], dst_n[sel]
        loc = d - c * per_core
        o = np.argsort(loc, kind="stable")
        sf, loc = sf[o], loc[o]
        tile_of = loc // 128
        for t in range(T):
            m = tile_of == t
            per_ct_srcf[c, t] = sf[m]
            per_ct_dloc[c, t] = loc[m]
            per_ct_slot[c, t] = loc[m] - t * 128
            counts[c, t] = m.sum()

    # per-tile chunk count: max over cores (SPMD shares one program)
    nch_t = [int(math.ceil(counts[:, t].max() / 128)) for t in range(T)]
    nchunk = max(nch_t)
    group = min(GROUP, nchunk)

    src16 = np.zeros((n_cores, T, 128, nchunk * 8), np.int16)
    onehot = np.zeros((n_cores, T, 128, nchunk * 128), bf16)
    onehotT = np.zeros((n_cores, T, 128, nchunk * 128), bf16)
    for c in range(n_cores):
        for t in range(T):
            e = counts[c, t]
            cap = nch_t[t] * 128
            ps = np.zeros(cap, np.int64)
            sl = np.full(cap, -1, np.int64)
            ps[:e] = per_ct_srcf[c, t]
            sl[:e] = per_ct_slot[c, t]
            src16[c, t, :, : nch_t[t] * 8] = _wrap16_rep(ps)
            # onehot[p, j*128 + d] = 1 iff edge (j*128+p) has dst slot d
            oh = np.zeros((128, nch_t[t], 128), np.float32)
            sl2 = sl.reshape(nch_t[t], 128)  # [j, p]
            jj, pp = np.nonzero(sl2 >= 0)
            oh[pp, jj, sl2[jj, pp]] = 1.0
            onehot[c, t, :, : cap] = oh.reshape(128, cap).astype(bf16)
            # onehotT[d, j*128 + e] = onehot[e, j*128 + d] (per-chunk transpose)
            onehotT[c, t, :, : cap] = (
                oh.transpose(2, 1, 0).reshape(128, cap).astype(bf16)
            )

    # permuted node features, transposed, padded rows, bf16, per core
    x_perm = np.zeros((n, f_in), np.float32)
    x_perm[perm] = x
    xT = []
    for c in range(n_cores):
        blk = np.zeros((rows_pad, f_in), np.float32)
        blk[:per_core] = x_perm[c * per_core : (c + 1) * per_core]
        xT.append(np.ascontiguousarray(blk.T).astype(bf16))

    g = lambda k: np.asarray(inputs[k], np.float32)

    # augmented weights: [W | ws | wd | Wl] with ws_h = W_h @ a_s_h etc.
    def aug(W, a_s, a_d, Wl):
        H, C = a_s.shape
        Wh = W.reshape(W.shape[0], H, C)
        ws = np.einsum("dhc,hc->dh", Wh, a_s)
        wd = np.einsum("dhc,hc->dh", Wh, a_d)
        return np.concatenate([W, ws, wd, Wl], 1).astype(bf16)

    waug1 = aug(g("W1"), g("a1s"), g("a1d"), g("Wl1"))   # [50, 2056]
    waug2 = aug(g("W2"), g("a2s"), g("a2d"), g("Wl2"))   # [1024, 2056]
    waug3 = aug(g("W3"), g("a3s"), g("a3d"), g("Wl3"))   # [1024, 859]

    rep = lambda v: np.ascontiguousarray(np.broadcast_to(v[None, :], (128, v.shape[0]))).astype(np.float32)
    base = dict(
        waug1=waug1, waug2=waug2, waug3=waug3,
        bsum1=rep(g("b1") + g("bl1")),
        bsum2=rep(g("b2") + g("bl2")),
        bsum3=rep(g("b3") + g("bl3")),
    )
    in_maps = []
    for c in range(n_cores):
        m = dict(base)
        m["xT1"] = xT[c]
        m["src16"] = src16[c]
        m["onehot"] = onehot[c]
        m["onehotT"] = onehotT[c]
        in_maps.append(m)

    h1, c1 = np.asarray(inputs["a1s"]).shape
    h3, c3 = np.asarray(inputs["a3s"]).shape
    cfg = dict(
        n=n, f_in=f_in, n_cores=n_cores, per_core=per_core,
        tiles_per_core=T, rows_last=rows_last, rows_pad=rows_pad,
        nchunk=nchunk, group=group, n_ag=n_ag, ag_rows=ag_rows,
        nch_t=nch_t,
        h1=h1, c1=c1, d1=h1 * c1, h3=h3, c3=c3,
    )
    return in_maps, cfg, perm


# --------------------------------------------------------------------------
# bass program
# --------------------------------------------------------------------------

def _layer_dims(cfg):
    """Static per-layer dims.
    waug cols: [h (H*C) | es (H) | ed (H) | lin (NLIN)]
    payload row (u16 units): [h bf16 (HC) | es f32 (2H u16)] padded to PW."""
    out = []
    for li in (1, 2, 3):
        if li < 3:
            h, c = cfg["h1"], cfg["c1"]
            din = cfg["f_in"] if li == 1 else cfg["d1"]
            nlin = cfg["d1"]
        else:
            h, c = cfg["h3"], cfg["c3"]
            din = cfg["d1"]
            nlin = cfg["c3"]
        hc = h * c
        naug = hc + 2 * h + nlin
        pw = math.ceil((hc + 2 * h) / 128) * 128       # u16; 2304B / 1536B rows
        esf = hc // 2                                   # f32 offset of es in payload
        kch = math.ceil(din / 128)
        out.append(dict(li=li, din=din, kch=kch, naug=naug, nlin=nlin,
                        h=h, c=c, hc=hc, pw=pw, esf=esf))
    return out


def _build(cfg):
    import concourse.bass as bass
    import concourse.bacc as bacc
    import concourse.mybir as mybir
    import concourse.tile as tile
    from contextlib import ExitStack

    f32 = mybir.dt.float32
    bf = mybir.dt.bfloat16
    i16 = mybir.dt.int16
    i32 = mybir.dt.int32
    u16 = mybir.dt.uint16
    EXP = mybir.ActivationFunctionType.Exp
    ALU = mybir.AluOpType

    n_cores = cfg["n_cores"]
    T = cfg["tiles_per_core"]
    per_core = cfg["per_core"]
    rows_pad = cfg["rows_pad"]
    NCHUNK = cfg["nchunk"]
    GRP = cfg["group"]
    NCH_T = cfg["nch_t"]
    NAG = cfg["n_ag"]
    AGR = cfg["ag_rows"]
    D1 = cfg["d1"]
    layers = _layer_dims(cfg)
    PWMAX = max(L["pw"] for L in layers)

    nc = bacc.Bacc(None, target_bir_lowering=False)

    # ---- parameters -----------------------------------------------------
    xT1 = nc.declare_dram_parameter("xT1", [cfg["f_in"], rows_pad], bf, isOutput=False)
    waug_p = {L["li"]: nc.declare_dram_parameter(f"waug{L['li']}", [L["din"], L["naug"]], bf, isOutput=False)
              for L in layers}
    bsum_p = {L["li"]: nc.declare_dram_parameter(f"bsum{L['li']}", [128, L["nlin"]], f32, isOutput=False)
              for L in layers}
    src16_p = nc.declare_dram_parameter("src16", [T, 128, NCHUNK * 8], i16, isOutput=False)
    onehot_p = nc.declare_dram_parameter("onehot", [T, 128, NCHUNK * 128], bf, isOutput=False)
    onehotT_p = nc.declare_dram_parameter("onehotT", [T, 128, NCHUNK * 128], bf, isOutput=False)
    out_p = nc.declare_dram_parameter("out", [per_core, cfg["c3"]], f32, isOutput=True)

    with tile.TileContext(nc, num_cores=n_cores) as tc, ExitStack() as ctx:
        # ---- dram scratch ----------------------------------------------
        dram = ctx.enter_context(tc.tile_pool(name="dram", bufs=1, space="DRAM"))
        pshard = {L["li"]: dram.tile([rows_pad, L["pw"]], u16, tag=f"pshard{L['li']}", name=f"pshard{L['li']}")
                  for L in layers}
        pfull = {L["li"]: dram.tile([n_cores * rows_pad, L["pw"]], u16, tag=f"pfull{L['li']}",
                                    name=f"pfull{L['li']}",
                                    **({} if CHUNKED_AG else dict(addr_space="Shared")))
                 for L in layers}
        linb = {L["li"]: dram.tile([rows_pad, L["nlin"]], f32, tag=f"lin{L['li']}", name=f"lin{L['li']}")
                for L in layers}

        # ---- sbuf pools -------------------------------------------------
        consts = ctx.enter_context(tc.tile_pool(name="consts", bufs=1))
        waugp = ctx.enter_context(tc.tile_pool(name="waugp", bufs=1))
        bsump = ctx.enter_context(tc.tile_pool(name="bsump", bufs=1))
        idxp = ctx.enter_context(tc.tile_pool(name="idxp", bufs=2))
        ohp = ctx.enter_context(tc.tile_pool(name="ohp", bufs=3))
        ohtp = ctx.enter_context(tc.tile_pool(name="ohtp", bufs=3))
        gp12 = ctx.enter_context(tc.tile_pool(name="gp12", bufs=2))
        gp3 = ctx.enter_context(tc.tile_pool(name="gp3", bufs=2))
        wep = ctx.enter_context(tc.tile_pool(name="wep", bufs=3))
        edap = ctx.enter_context(tc.tile_pool(name="edap", bufs=1))
        rhsp = ctx.enter_context(tc.tile_pool(name="rhsp", bufs=3))
        xtp = ctx.enter_context(tc.tile_pool(name="xtp", bufs=2))
        ptp = ctx.enter_context(tc.tile_pool(name="ptp", bufs=2))
        ltp = ctx.enter_context(tc.tile_pool(name="ltp", bufs=2))
        epip = ctx.enter_context(tc.tile_pool(name="epip", bufs=2))
        recp = ctx.enter_context(tc.tile_pool(name="recp", bufs=2))
        # psum (8 banks): big 2 (pfeat agg / pdL dense, disjoint lifetimes)
        # + pdF 2 + psmall 2x1 + edm 2x1
        pfeatp = ctx.enter_context(tc.tile_pool(name="pfeatp", bufs=1, space="PSUM"))
        psmallp = ctx.enter_context(tc.tile_pool(name="psmallp", bufs=2, space="PSUM"))
        pdFp = ctx.enter_context(tc.tile_pool(name="pdFp", bufs=1, space="PSUM"))
        edmp = ctx.enter_context(tc.tile_pool(name="edmp", bufs=2, space="PSUM"))

        # ---- constants --------------------------------------------------
        iota_c = consts.tile([128, 128], i32, tag="iota_c")
        nc.gpsimd.iota(iota_c[:, :], pattern=[[1, 128]], base=0, channel_multiplier=0)
        iota_p = consts.tile([128, 1], i32, tag="iota_p")
        nc.gpsimd.iota(iota_p[:, :], pattern=[[1, 1]], base=0, channel_multiplier=1)
        iota_cf = consts.tile([128, 128], f32, tag="iota_cf")
        nc.vector.tensor_copy(iota_cf[:, :], iota_c[:, :])
        iota_pf = consts.tile([128, 1], f32, tag="iota_pf")
        nc.vector.tensor_copy(iota_pf[:, :], iota_p[:, :])
        ident = consts.tile([128, 128], bf, tag="ident")
        nc.vector.tensor_scalar(out=ident[:, :], in0=iota_cf[:, :],
                                scalar1=iota_pf[:, 0:1], scalar2=None, op0=ALU.is_equal)

        # ---- layer constants (all preloaded; ~51KB/partition) -----------
        wt = {}
        for L in layers:
            li, KCH, DIN, NAUG = L["li"], L["kch"], L["din"], L["naug"]
            wt[li] = [waugp.tile([128, NAUG], bf, tag=f"waug{li}_k{k}", name=f"waug{li}_k{k}")
                      for k in range(KCH)]
            for k in range(KCH):
                kk = min(128, DIN - k * 128)
                nc.sync.dma_start(out=wt[li][k][:kk, :], in_=waug_p[li][k * 128 : k * 128 + kk, :])
        bsum = {}
        for L in layers:
            li = L["li"]
            bsum[li] = bsump.tile([128, L["nlin"]], f32, tag=f"bsum{li}", name=f"bsum{li}")
            nc.sync.dma_start(out=bsum[li][:, :], in_=bsum_p[li][:, :])
        # per-layer resident ed tables: [128 dst-slot, T*H] (written by dense)
        ed_all = {}
        for L in layers:
            li, H = L["li"], L["h"]
            ed_all[li] = edap.tile([128, T * H], bf, tag=f"ed_all{li}", name=f"ed_all{li}")

        # -----------------------------------------------------------------
        def emit_dense(L, t, lhsT_of):
            """Dense phase for tile t of layer L: psum [h|esd|lin] via matmuls,
            then payload/edtab/linb stores. lhsT_of(k, kk) -> [kk,128] bf AP."""
            li, KCH, DIN = L["li"], L["kch"], L["din"]
            H, HC, NLIN, PW = L["h"], L["hc"], L["nlin"], L["pw"]
            pdF = pdFp.tile([128, 1024], f32, tag="pdF", name="pdF")
            pdE = psmallp.tile([128, 128], f32, tag="ps", name="pdE")
            pdL = pfeatp.tile([128, 1024], f32, tag="pfeat", name="pdL")
            for k in range(KCH):
                kk = min(128, DIN - k * 128)
                lhsT = lhsT_of(k, kk)
                st, sp = (k == 0), (k == KCH - 1)
                for nb in range(math.ceil(HC / 512)):
                    w = min(512, HC - nb * 512)
                    nc.tensor.matmul(pdF[:, nb * 512 : nb * 512 + w], lhsT,
                                     wt[li][k][:kk, nb * 512 : nb * 512 + w], start=st, stop=sp)
                nc.tensor.matmul(pdE[:, : 2 * H], lhsT,
                                 wt[li][k][:kk, HC : HC + 2 * H], start=st, stop=sp)
                for nb in range(math.ceil(NLIN / 512)):
                    w = min(512, NLIN - nb * 512)
                    nc.tensor.matmul(pdL[:, nb * 512 : nb * 512 + w], lhsT,
                                     wt[li][k][:kk, HC + 2 * H + nb * 512 : HC + 2 * H + nb * 512 + w],
                                     start=st, stop=sp)
            # payload: [h bf16 | es f32 | pad]
            pt = ptp.tile([128, PWMAX], u16, tag="pt")
            ptb = pt.bitcast(bf)
            nc.vector.tensor_copy(ptb[:, :HC], pdF[:, :HC])
            ptf = pt.bitcast(f32)
            nc.vector.tensor_copy(ptf[:, L["esf"] : L["esf"] + H], pdE[:, :H])
            if HC + 2 * H < PW:
                nc.vector.memset(pt[:, HC + 2 * H : PW], 0.0)
            nc.sync.dma_start(out=pshard[li][t * 128 : (t + 1) * 128, :], in_=pt[:, :PW])
            # ed -> resident SBUF table for this layer (row = dst slot of tile t)
            nc.vector.tensor_copy(ed_all[li][:, t * H : (t + 1) * H], pdE[:, H : 2 * H])
            # lin + bias
            lt = ltp.tile([128, 1024], f32, tag="lt")
            nc.vector.tensor_tensor(out=lt[:, :NLIN], in0=pdL[:, :NLIN], in1=bsum[li][:, :], op=ALU.add)
            nc.sync.dma_start(out=linb[li][t * 128 : (t + 1) * 128, :], in_=lt[:, :NLIN])

        def emit_ag(L, k):
            """Chunked AllGather k of layer L's payload shard."""
            li = L["li"]
            if not CHUNKED_AG:
                if k == NAG - 1:  # single full AllGather once all tiles stored
                    nc.gpsimd.collective_compute(
                        "AllGather", ALU.bypass,
                        replica_groups=[list(range(n_cores))],
                        ins=[pshard[li][:, :]],
                        outs=[pfull[li][:, :]],
                    )
                return
            a, b = k * AGR, (k + 1) * AGR
            nc.gpsimd.collective_compute(
                "AllGather", ALU.bypass,
                replica_groups=[list(range(n_cores))],
                ins=[pshard[li][a:b, :]],
                outs=[pfull[li][k * n_cores * AGR : (k + 1) * n_cores * AGR, :]],
            )

        # ---------------- phase 1: dense layer 1 -------------------------
        L1 = layers[0]
        f_in = cfg["f_in"]
        for t in range(T):
            x1t = xtp.tile([128, 128], bf, tag="xT_k0", name="x1t")
            nc.sync.dma_start(out=x1t[:f_in, :], in_=xT1[:, t * 128 : (t + 1) * 128])
            emit_dense(L1, t, lambda k, kk: x1t[:f_in, :])
            if (t + 1) % AG_TILES == 0:
                emit_ag(L1, t // AG_TILES)

        # ---------------- phases 2..4: agg(li) [+ dense(li+1)] -----------
        for L in layers:
            li = L["li"]
            H, C, HC, PW, ESF, NLIN = L["h"], L["c"], L["hc"], L["pw"], L["esf"], L["nlin"]
            Lnext = layers[li] if li < 3 else None
            gpool = gp12 if li < 3 else gp3

            for t in range(T):
                # deferred AG trigger for the next layer's payload
                if Lnext is not None and t >= AG_TILES - 1 + AG_LAG and (t - AG_LAG + 1) % AG_TILES == 0:
                    emit_ag(Lnext, (t - AG_LAG + 1) // AG_TILES - 1)

                NCH = NCH_T[t]
                NGt = math.ceil(NCH / GRP)
                s16 = idxp.tile([128, NCHUNK * 8], i16, tag="s16")
                nc.sync.dma_start(out=s16[:, : NCH * 8], in_=src16_p[t][:, : NCH * 8])
                lt2 = ltp.tile([128, 1024], f32, tag="lt2")
                nc.sync.dma_start(out=lt2[:, :NLIN], in_=linb[li][t * 128 : (t + 1) * 128, :])

                pfeat = pfeatp.tile([128, 1024], f32, tag="pfeat", name="pfeat")
                pden = psmallp.tile([128, 128], f32, tag="ps", name="pden")

                for g in range(NGt):
                    gr = min(GRP, NCH - g * GRP)  # chunks in this group
                    G = gpool.tile([128, GRP, PW], u16, tag=f"G{li if li == 3 else 12}")
                    nc.gpsimd.dma_gather(
                        out_ap=G[:, :gr, :], in_ap=pfull[li][:, :],
                        idxs_ap=s16[:, g * GRP * 8 : (g * GRP + gr) * 8],
                        num_idxs=gr * 128, num_idxs_reg=gr * 128, elem_size=PW,
                    )
                    oh = ohp.tile([128, GRP, 128], bf, tag="oh")
                    nc.sync.dma_start(
                        out=oh[:, :gr, :],
                        in_=onehot_p[t][:, g * GRP * 128 : (g * GRP + gr) * 128]
                        .rearrange("p (g c) -> p g c", g=gr),
                    )
                    oht = ohtp.tile([128, GRP, 128], bf, tag="oht")
                    nc.sync.dma_start(
                        out=oht[:, :gr, :],
                        in_=onehotT_p[t][:, g * GRP * 128 : (g * GRP + gr) * 128]
                        .rearrange("p (g c) -> p g c", g=gr),
                    )
                    # ed broadcast to edges via PE: edm[e,h] = sum_d ohT[d,e] ed[d,h]
                    edps = edmp.tile([128, 64], f32, tag="edm")
                    for cch in range(gr):
                        nc.tensor.matmul(edps[:, cch * H : (cch + 1) * H],
                                         oht[:, cch, :], ed_all[li][:, t * H : (t + 1) * H],
                                         start=(cch == 0), stop=(cch == gr - 1))
                    edm = edps[:, : gr * H].rearrange("p (g h) -> p g h", g=gr)
                    Gf = G.bitcast(f32)
                    Gb = G.bitcast(bf)
                    tl = wep.tile([128, GRP, H], f32, tag="tl")
                    wf = wep.tile([128, GRP, H], f32, tag="wf")
                    web = wep.tile([128, GRP, H], bf, tag="web")
                    nc.vector.tensor_tensor(out=tl[:, :gr, :], in0=Gf[:, :gr, ESF : ESF + H],
                                            in1=edm, op=ALU.add)
                    # w = exp(max(t, 0.2t)) == exp(leakyrelu(t)); |t| <~ 8
                    nc.vector.tensor_scalar(out=wf[:, :gr, :], in0=tl[:, :gr, :],
                                            scalar1=0.2, scalar2=None, op0=ALU.mult)
                    nc.vector.tensor_tensor(out=wf[:, :gr, :], in0=tl[:, :gr, :],
                                            in1=wf[:, :gr, :], op=ALU.max)
                    nc.scalar.activation(web[:, :gr, :], wf[:, :gr, :], EXP)
                    for cch in range(gr):
                        j = g * GRP + cch
                        rq = rhsp.tile([128, 1024], bf, tag="rq")
                        nc.vector.tensor_tensor(
                            out=rq[:, :HC].rearrange("p (h c) -> p h c", h=H),
                            in0=Gb[:, cch, :HC].rearrange("p (h c) -> p h c", h=H),
                            in1=web[:, cch, :, None].broadcast_to((128, H, C)),
                            op=ALU.mult,
                        )
                        st, sp = (j == 0), (j == NCH - 1)
                        lhsT = oh[:, cch, :]
                        for nb in range(math.ceil(HC / 512)):
                            w = min(512, HC - nb * 512)
                            nc.tensor.matmul(pfeat[:, nb * 512 : nb * 512 + w], lhsT,
                                             rq[:, nb * 512 : nb * 512 + w], start=st, stop=sp)
                        nc.tensor.matmul(pden[:, :H], lhsT, web[:, cch, :], start=st, stop=sp)

                # ---- epilogue ----
                rec = recp.tile([128, 16], f32, tag="rec")
                nc.vector.tensor_scalar(out=rec[:, 8 : 8 + H], in0=pden[:, :H],
                                        scalar1=1e-16, scalar2=None, op0=ALU.max)
                nc.vector.reciprocal(rec[:, :H], rec[:, 8 : 8 + H])
                if li == 3:
                    nc.vector.tensor_scalar(out=rec[:, :H], in0=rec[:, :H],
                                            scalar1=1.0 / H, scalar2=None, op0=ALU.mult)
                xt = epip.tile([128, 1024], f32, tag="xt")
                nc.vector.tensor_tensor(
                    out=xt[:, :HC].rearrange("p (h c) -> p h c", h=H),
                    in0=pfeat[:, :HC].rearrange("p (h c) -> p h c", h=H),
                    in1=rec[:, :H, None].broadcast_to((128, H, C)),
                    op=ALU.mult,
                )
                r = 128 if t < T - 1 else cfg["rows_last"]
                if li < 3:
                    s = epip.tile([128, 1024], f32, tag="s")
                    u = epip.tile([128, 1024], f32, tag="u")
                    v = epip.tile([128, 1024], f32, tag="v")
                    e = epip.tile([128, 1024], f32, tag="e")
                    xo = epip.tile([128, 1024], bf, tag="xo")
                    nc.vector.tensor_tensor(out=s[:, :], in0=xt[:, :], in1=lt2[:, :], op=ALU.add)
                    nc.vector.tensor_scalar(out=u[:, :], in0=s[:, :], scalar1=0.0, scalar2=None, op0=ALU.min)
                    nc.scalar.activation(e[:, :], u[:, :], EXP)
                    nc.vector.tensor_scalar(out=v[:, :], in0=s[:, :], scalar1=0.0, scalar2=-1.0,
                                            op0=ALU.max, op1=ALU.add)
                    nc.vector.tensor_tensor(out=xo[:, :], in0=v[:, :], in1=e[:, :], op=ALU.add)
                    # ---- fused dense of next layer: PE-transpose lhsT ----
                    xTt = [xtp.tile([128, 128], bf, tag=f"xT_k{k}", name=f"xT_k{k}") for k in range(8)]
                    for k in range(8):
                        tp = psmallp.tile([128, 128], f32, tag="ps", name=f"tp{k}")
                        tpb = tp.bitcast(bf)
                        nc.tensor.transpose(tpb[:, :128], xo[:, k * 128 : (k + 1) * 128], ident[:, :])
                        nc.vector.tensor_copy(xTt[k][:, :], tpb[:, :128])
                    emit_dense(Lnext, t, lambda k, kk: xTt[k][:, :])
                else:
                    x3 = xt[:, :HC].rearrange("p (h c) -> p h c", h=H)
                    m1 = epip.tile([128, 3 * 128], f32, tag="m1")
                    m1v = m1[:, : 3 * C].rearrange("p (h c) -> p h c", h=3)
                    nc.vector.tensor_tensor(out=m1v, in0=x3[:, 0:3, :], in1=x3[:, 3:6, :], op=ALU.add)
                    ot = epip.tile([128, 128], f32, tag="ot")
                    nc.vector.tensor_tensor(out=ot[:, :C], in0=m1[:, :C], in1=m1[:, C : 2 * C], op=ALU.add)
                    nc.vector.tensor_tensor(out=ot[:, :C], in0=ot[:, :C], in1=m1[:, 2 * C : 3 * C], op=ALU.add)
                    nc.vector.tensor_tensor(out=ot[:, :C], in0=ot[:, :C], in1=lt2[:, :C], op=ALU.add)
                    nc.sync.dma_start(out=out_p[t * 128 : t * 128 + r, :], in_=ot[:r, :C])

            # tail AG chunks for the next layer not yet emitted
            if Lnext is not None:
                for k in range((T - AG_LAG) // AG_TILES, NAG):
                    emit_ag(Lnext, k)

    nc.finalize()
    return nc


# --------------------------------------------------------------------------
# runner
# --------------------------------------------------------------------------

def _run(inputs, sim=False, trace=False, n_cores=N_CORES, tmpdir=None):
    in_maps, cfg, perm = _host_prep(inputs, n_cores)
    nc = _build(cfg)
    if sim:
        import concourse.bass_interp as bass_interp

        msim = bass_interp.MultiCoreSim(nc, n_cores)
        for c in range(n_cores):
            for k, v in in_maps[c].items():
                msim.cores[c].tensor(k)[:] = v
        msim.simulate()
        outs = [np.array(msim.cores[c].mem_tensor("out")) for c in range(n_cores)]
        exec_ns = None
    else:
        from concourse.bass_utils import run_bass_kernel_spmd

        res = run_bass_kernel_spmd(
            nc, in_maps, list(range(n_cores)), trace=trace, tmpdir=tmpdir
        )
        outs = [res.results[c]["out"] for c in range(n_cores)]
        exec_ns = res.exec_time_ns
    out_new = np.concatenate(outs, 0)
    out = np.empty_like(out_new)
    out[...] = out_new[perm]
    return out.astype(np.float32), exec_ns


def kernel(**inputs) -> np.ndarray:
    out, _ = _run(inputs)
    return out
